# revision 18
# baseline (speedup 1.0000x reference)
"""Trainium2 Bass kernel for ErnieLayout self-attention (B=4,S=1024,H=768,NH=12,HD=64).

Sharding: 8 cores = 4 batches x 2 head-groups (6 heads each).

Key structure (v2):
- Mask compaction: attention_mask zeroes ~50% of k columns EXACTLY
  (exp(s-1e10) == 0 in f32), so the host gathers the unmasked k positions
  (n_b ~ 485..541 for the fixed inputs) and pads to K=640. Scores / exp /
  PV / rel DMA all shrink by ~40%. Pad k rows carry rel = -15000 so
  exp(score) == 0 exactly; no mask tensor on device at all.
- Host does layout-only prep: hs pre-transposed (hsT), hs compacted for
  k/v (hskvT), rel_pos/rel_2d_pos gathered along k and transposed to
  [k, q] layout, all bf16. All arithmetic (projections, rel add, exp,
  matmuls, softmax normalization) stays on device.
- Scores computed TRANSPOSED ([k,q]): per 128-k chunk, r12 = r1+r2 (DVE)
  is injected into PSUM via ONE identity matmul (start=True), then the
  zero-padded-head kTz @ qT matmul accumulates on top (stop=True).
  exp has no bias and reads PSUM directly.
- Softmax denominator from the [V|ones] PV matmul; context transposed
  back per 128-q block on PE, scaled by the reciprocal on DVE, output
  accumulated in SBUF and written as two large bf16 DMAs in a
  hardware-friendly [qh, i, p, cols] layout the host un-permutes.
"""
import os
import numpy as np
import ml_dtypes

from concourse import bacc, mybir, tile
from concourse.bass_utils import run_bass_kernel_spmd
from concourse.masks import make_identity

B, S, H = 4, 1024, 768
NH, HD = 12, 64
N_CORES = 8
HPC = 6            # heads per core
COLS = HPC * HD    # 384 output columns per core
KC = H // 128      # 6 contraction chunks for projections
K = 640            # compacted+padded k length
SC = K // 128      # 5 k chunks
QH = 2             # q halves of 512
PAD_REL = -15000.0  # per-rel pad value; r1+r2 = -30000 -> exp == 0
bf16 = mybir.dt.bfloat16
f32 = mybir.dt.float32
i32 = mybir.dt.int32
AF = mybir.ActivationFunctionType
BF16_NP = ml_dtypes.bfloat16

_compiled = None
last_result = None  # BassKernelResults of the most recent run (for test harness)


def _build():
    nc = bacc.Bacc("TRN2", target_bir_lowering=False, debug=False,
                   num_devices=N_CORES)
    hsT = nc.dram_tensor("hsT", [H, S], bf16, kind="ExternalInput").ap()
    hskvT = nc.dram_tensor("hskvT", [H, K], bf16, kind="ExternalInput").ap()
    wq = nc.dram_tensor("wq", [H, COLS], bf16, kind="ExternalInput").ap()
    wk = nc.dram_tensor("wk", [H, COLS], bf16, kind="ExternalInput").ap()
    wv = nc.dram_tensor("wv", [H, COLS], bf16, kind="ExternalInput").ap()
    bq = nc.dram_tensor("bq", [COLS], f32, kind="ExternalInput").ap()
    bk = nc.dram_tensor("bk", [COLS], f32, kind="ExternalInput").ap()
    bv = nc.dram_tensor("bv", [COLS], f32, kind="ExternalInput").ap()
    rel1 = nc.dram_tensor("rel1", [HPC, K, S], bf16, kind="ExternalInput").ap()
    rel2 = nc.dram_tensor("rel2", [HPC, K, S], bf16, kind="ExternalInput").ap()
    # out[qh, p, i, c] = ctx[qh*512 + i*128 + p, c]
    out = nc.dram_tensor("out", [QH, 128, 4, COLS], bf16,
                         kind="ExternalOutput").ap()

    with tile.TileContext(nc) as tc:
        with tc.tile_pool(name="const", bufs=1) as const, \
             tc.tile_pool(name="hst", bufs=1) as hst_pool, \
             tc.tile_pool(name="w", bufs=1) as w_pool, \
             tc.tile_pool(name="qk", bufs=1) as qk_pool, \
             tc.tile_pool(name="v", bufs=1) as v_pool, \
             tc.tile_pool(name="r1", bufs=2) as r1_pool, \
             tc.tile_pool(name="r2", bufs=2) as r2_pool, \
             tc.tile_pool(name="r12", bufs=3) as r12_pool, \
             tc.tile_pool(name="et", bufs=3) as e_pool, \
             tc.tile_pool(name="ctxt", bufs=2) as ctxt_pool, \
             tc.tile_pool(name="ob", bufs=1) as ob_pool:

            # ---- startup DMAs spread over the 3 issue-capable queues
            # (sync / scalar / gpsimd), each ordered by first use. The
            # scalar+gpsimd issues cost ~1us each, so no single queue may
            # serialize the critical path. ----
            import concourse.bass as bass
            hsT_sb = hst_pool.tile([128, KC, S], bf16)
            _hsT_r = hsT.rearrange("(c p) q -> p c q", p=128)
            hskvT_sb = hst_pool.tile([128, KC, K], bf16)
            _hskvT_r = hskvT.rearrange("(c p) q -> p c q", p=128)
            wq_sb = w_pool.tile([128, KC, COLS], bf16)
            wk_sb = w_pool.tile([128, KC, COLS], bf16)
            wv_sb = w_pool.tile([128, KC, COLS], bf16)
            _wq_r = wq.rearrange("(c p) n -> p c n", p=128)
            _wk_r = wk.rearrange("(c p) n -> p c n", p=128)

            # gpsimd ring: qproj/vproj weights
            nc.gpsimd.dma_start(out=wq_sb[:, :, 0:128], in_=_wq_r[:, :, 0:128])
            nc.gpsimd.dma_start(out=wv_sb, in_=wv.rearrange("(c p) n -> p c n", p=128))
            for c2 in range(3):
                nc.gpsimd.dma_start(out=hskvT_sb[:, c2 * 2:(c2 + 1) * 2, :],
                                    in_=_hskvT_r[:, c2 * 2:(c2 + 1) * 2, :])
            bv_bc = bass.AP(tensor=bv.tensor, offset=bv.offset,
                            ap=[[0, 128]] + list(bv.ap))
            bv_sb = const.tile([128, COLS], f32)
            nc.gpsimd.dma_start(out=bv_sb, in_=bv_bc)
            nc.gpsimd.dma_start(out=wq_sb[:, :, 128:384], in_=_wq_r[:, :, 128:384])

            # scalar ring: biases, hsT, kproj weights
            bq_sb = const.tile([128, 3], f32)
            nc.scalar.dma_start(out=bq_sb, in_=bq.rearrange("(c p) -> p c", p=128))
            bk_sb = const.tile([128, 3], f32)
            nc.scalar.dma_start(out=bk_sb, in_=bk.rearrange("(c p) -> p c", p=128))
            for c2 in range(3):
                nc.scalar.dma_start(out=hsT_sb[:, c2 * 2:(c2 + 1) * 2, :],
                                    in_=_hsT_r[:, c2 * 2:(c2 + 1) * 2, :])
            nc.scalar.dma_start(out=wk_sb[:, :, 0:128], in_=_wk_r[:, :, 0:128])
            nc.scalar.dma_start(out=wk_sb[:, :, 128:384], in_=_wk_r[:, :, 128:384])

            ident_f32 = const.tile([128, 128], f32)
            make_identity(nc, ident_f32)
            ident_b = const.tile([128, 128], bf16)
            nc.vector.tensor_copy(ident_b, ident_f32)

            _psum_cms = [tc.tile_pool(name="psA", bufs=2, space="PSUM"),
                         tc.tile_pool(name="psS", bufs=3, space="PSUM"),
                         tc.tile_pool(name="psV", bufs=2, space="PSUM"),
                         tc.tile_pool(name="psT", bufs=1, space="PSUM")]
            proj_psum, sc_psum, pv_psum, pt_psum = (cm.__enter__()
                                                    for cm in _psum_cms)

            # HAM warmup: dependency-free matmuls run during the startup DMA
            # window, flipping the PE clock gate to 2.4GHz before the real
            # projections arrive.
            garbage = const.tile([128, 384], bf16)
            nc.vector.memset(garbage, 0.0)
            def warm_burst(n):
                warm = sc_psum.tile([128, 512], f32, tag="ps")
                for _ in range(n):
                    nc.tensor.matmul(warm[:, 0:256], garbage[:, 0:128],
                                     garbage[:, 128:384], start=True, stop=True)
            warm_burst(44)

            # qT: [d(2 heads stacked), q] per head-pair hp; q scaled by 1/8.
            # kTz zero-padded per head (kTz[:, hp, hi]: head hi's 64 d-rows
            # live at their stacked position, other 64 rows are 0) so the
            # scores matmul streams a full 128-partition lhsT.
            qT = qk_pool.tile([128, 3, S], bf16)
            kTz = qk_pool.tile([128, 3, 2, K], bf16)
            nc.vector.memset(kTz, 0.0)
            v_sb = v_pool.tile([128, SC, HPC, HD + 1], bf16)
            nc.gpsimd.memset(v_sb[:, :, :, HD], 1.0)
            out_sb = ob_pool.tile([128, QH, 4, COLS], bf16)

            def emit_proj_q(hp):
                for sh in range(QH):
                    ssl = slice(sh * 512, (sh + 1) * 512)
                    psq = proj_psum.tile([128, 512], f32, tag="proj")
                    for c in range(KC):
                        nc.tensor.matmul(psq, wq_sb[:, c, hp * 128:(hp + 1) * 128],
                                         hsT_sb[:, c, ssl],
                                         start=(c == 0), stop=(c == KC - 1))
                    nc.scalar.activation(out=qT[:, hp, ssl], in_=psq, func=AF.Identity,
                                         bias=bq_sb[:, hp:hp + 1], scale=0.125)

            def emit_proj_k(hp):
                for part, ksl in ((0, slice(0, 512)), (1, slice(512, K))):
                    kw = 512 if part == 0 else K - 512
                    psk = proj_psum.tile([128, 512], f32, tag="proj")
                    for c in range(KC):
                        nc.tensor.matmul(psk[:, 0:kw],
                                         wk_sb[:, c, hp * 128:(hp + 1) * 128],
                                         hskvT_sb[:, c, ksl],
                                         start=(c == 0), stop=(c == KC - 1))
                    nc.scalar.activation(out=kTz[0:64, hp, 0, ksl],
                                         in_=psk[0:64, 0:kw], func=AF.Identity,
                                         bias=bk_sb[0:64, hp:hp + 1], scale=1.0)
                    nc.scalar.activation(out=kTz[64:128, hp, 1, ksl],
                                         in_=psk[64:128, 0:kw], func=AF.Identity,
                                         bias=bk_sb[64:128, hp:hp + 1], scale=1.0)

            def emit_proj_v(scs):
                for sc in scs:
                    psv_full = proj_psum.tile([128, 512], f32, tag="proj")
                    psv = psv_full[:, 0:384]
                    for c in range(KC):
                        nc.tensor.matmul(psv, hskvT_sb[:, c, sc * 128:(sc + 1) * 128],
                                         wv_sb[:, c, :],
                                         start=(c == 0), stop=(c == KC - 1))
                    nc.vector.tensor_add(
                        v_sb[:, sc, :, 0:HD],
                        psv.rearrange("p (h d) -> p h d", h=HPC),
                        bv_sb.rearrange("p (h d) -> p h d", h=HPC))

            def emit_rel_load(h, qh):
                # per-(head, q-half) loads so unit (h, qh) only waits on its
                # own 1.3MB, not the whole head's 2.6MB
                qsl = slice(qh * 512, (qh + 1) * 512)
                r1t = r1_pool.tile([128, SC, 512], bf16, tag="r1")
                nc.sync.dma_start(
                    out=r1t, in_=rel1[h].rearrange("(c p) q -> p c q", p=128)[:, :, qsl])
                r2t = r2_pool.tile([128, SC, 512], bf16, tag="r2")
                nc.sync.dma_start(
                    out=r2t, in_=rel2[h].rearrange("(c p) q -> p c q", p=128)[:, :, qsl])
                return (r1t, r2t)

            def emit_preadd(rel_t):
                r1t, r2t = rel_t
                r12 = r12_pool.tile([128, SC, 512], bf16, tag="r12")
                nc.vector.tensor_add(r12, r1t, r2t)
                return r12

            def emit_scores(h, qh, r12):
                hp, hi = divmod(h, 2)
                qsl = slice(qh * 512, (qh + 1) * 512)
                et = e_pool.tile([128, SC, 512], bf16, tag="et")
                for kc in range(SC):
                    ps = sc_psum.tile([128, 512], f32, tag="ps")
                    nc.tensor.matmul(ps, ident_b, r12[:, kc, :],
                                     start=True, stop=False)
                    nc.tensor.matmul(ps, kTz[:, hp, hi, kc * 128:(kc + 1) * 128],
                                     qT[:, hp, qsl], start=False, stop=True)
                    nc.scalar.activation(out=et[:, kc, :], in_=ps, func=AF.Exp)
                return (h, qh, et)

            def emit_pv(state):
                h, qh, et = state
                pv = pv_psum.tile([HD + 1, 512], f32, tag="pv")
                for kc in range(SC):
                    nc.tensor.matmul(pv, v_sb[:, kc, h, :], et[:, kc, :],
                                     start=(kc == 0), stop=(kc == SC - 1))
                ctxT = ctxt_pool.tile([HD + 1, 512], bf16, tag="ctxT")
                nc.vector.tensor_copy(ctxT, pv)
                return (h, qh, ctxT)

            def emit_attn_out(state):
                h, qh, ctxT = state
                pt = pt_psum.tile([128, 4, HD + 2], bf16, tag="pt")
                for i in range(4):
                    nc.tensor.matmul(pt[:, i, 0:HD + 1], ctxT[:, i * 128:(i + 1) * 128],
                                     ident_b[:HD + 1, :HD + 1],
                                     is_transpose=True, start=True, stop=True)
                rec = ob_pool.tile([128, 4], f32, tag="rec")
                nc.vector.reciprocal(rec, pt[:, :, HD])
                rec_bc = bass.AP(tensor=rec.tensor, offset=rec.offset,
                                 ap=[list(rec.ap[0]), [rec.ap[1][0], 4], [0, HD]])
                nc.vector.tensor_mul(out_sb[:, qh, :, h * HD:(h + 1) * HD],
                                      pt[:, :, 0:HD], rec_bc)

            # software pipeline: scores(u) -> pv(u-1) -> attn_out(u-2).
            # rel loads run two units ahead, preadds one ahead. vproj is
            # deferred until after unit-0/1 scores so unit-0's exp isn't
            # queued behind 30 vproj matmuls at startup.
            units = [(h, qh) for h in range(HPC) for qh in range(QH)]
            et_q = []
            pending = []
            rel_q = []
            r12_q = []

            def step(idx, skip_preadd=False):
                if idx + 2 < len(units):
                    rel_q.append(emit_rel_load(*units[idx + 2]))
                if idx + 1 < len(units) and not skip_preadd:
                    r12_q.append(emit_preadd(rel_q.pop(0)))
                et_q.append(emit_scores(*units[idx], r12_q.pop(0)))
                if len(et_q) > 1:
                    pending.append(emit_pv(et_q.pop(0)))
                if len(pending) > 1:
                    emit_attn_out(pending.pop(0))

            rel_q.append(emit_rel_load(*units[0]))
            rel_q.append(emit_rel_load(*units[1]))
            emit_proj_q(0)
            warm_burst(8)
            emit_proj_k(0)
            r12_q.append(emit_preadd(rel_q.pop(0)))
            r12_q.append(emit_preadd(rel_q.pop(0)))
            warm_burst(8)
            step(0, skip_preadd=True)
            emit_proj_v(range(SC))
            step(1, skip_preadd=True)
            emit_proj_q(1)
            emit_proj_k(1)
            r12_q.append(emit_preadd(rel_q.pop(0)))
            step(2)
            emit_proj_q(2)
            emit_proj_k(2)
            for idx in range(3, len(units)):
                step(idx)
            pending.append(emit_pv(et_q.pop(0)))
            emit_attn_out(pending.pop(0))
            nc.sync.dma_start(out=out[0], in_=out_sb[:, 0, :, :])
            emit_attn_out(pending.pop(0))
            nc.sync.dma_start(out=out[1], in_=out_sb[:, 1, :, :])

            for cm in reversed(_psum_cms):
                cm.__exit__(None, None, None)

    nc.compile()
    return nc


def _get_compiled():
    global _compiled
    if _compiled is None:
        _compiled = _build()
    return _compiled


def kernel(hidden_states, Wq, bq, Wk, bk, Wv, bv, rel_pos, rel_2d_pos,
           attention_mask, _trace=False):
    global last_result
    nc = _get_compiled()

    hidden_states = np.asarray(hidden_states, np.float32)
    Wq, Wk, Wv = (np.asarray(w, np.float32) for w in (Wq, Wk, Wv))
    bq, bk, bv = (np.asarray(x, np.float32) for x in (bq, bk, bv))
    rel_pos = np.asarray(rel_pos, np.float32)
    rel_2d_pos = np.asarray(rel_2d_pos, np.float32)
    attention_mask = np.asarray(attention_mask, np.int32)

    # per-batch unmasked k indices (mask==0 attends; mask==1 scores -1e10
    # -> exp == 0 exactly, so masked columns are dropped exactly)
    idxs = []
    for b in range(B):
        idx = np.nonzero(attention_mask[b, 0, 0] == 0)[0]
        assert len(idx) <= K, f"unmasked count {len(idx)} exceeds K={K}"
        idxs.append(idx)

    in_maps = []
    for c in range(N_CORES):
        b, hg = divmod(c, 2)
        cs = slice(hg * COLS, (hg + 1) * COLS)
        h0 = hg * HPC
        idx = idxs[b]
        n = len(idx)

        hsT_np = np.ascontiguousarray(hidden_states[b].T).astype(BF16_NP)
        hskvT_np = np.zeros((H, K), BF16_NP)
        hskvT_np[:, :n] = hidden_states[b][idx].T
        rel1_np = np.full((HPC, K, S), PAD_REL, BF16_NP)
        rel2_np = np.full((HPC, K, S), PAD_REL, BF16_NP)
        for j in range(HPC):
            rel1_np[j, :n] = rel_pos[b, h0 + j][:, idx].T
            rel2_np[j, :n] = rel_2d_pos[b, h0 + j][:, idx].T

        in_maps.append({
            "hsT": hsT_np,
            "hskvT": hskvT_np,
            "wq": Wq[:, cs].astype(BF16_NP),
            "wk": Wk[:, cs].astype(BF16_NP),
            "wv": Wv[:, cs].astype(BF16_NP),
            "bq": np.ascontiguousarray(bq[cs]) * np.float32(0.125),
            "bk": np.ascontiguousarray(bk[cs]),
            "bv": np.ascontiguousarray(bv[cs]),
            "rel1": rel1_np,
            "rel2": rel2_np,
        })

    kwargs = {}
    if _trace or os.environ.get("KERNEL_TRACE"):
        kwargs["trace"] = True
    last_result = run_bass_kernel_spmd(nc, in_maps, list(range(N_CORES)), **kwargs)

    result = np.empty((B, S, H), np.float32)
    for c in range(N_CORES):
        b, hg = divmod(c, 2)
        # out[qh, p, i, c] -> ctx[qh*512 + i*128 + p, c]
        o = np.asarray(last_result.results[c]["out"], BF16_NP).astype(np.float32)
        o = o.transpose(0, 2, 1, 3).reshape(S, COLS)
        result[b, :, hg * COLS:(hg + 1) * COLS] = o
    return result


# revision 20
# speedup vs baseline: 1.1439x; 1.1439x over previous
"""Trainium2 Bass kernel for ErnieLayout self-attention (B=4,S=1024,H=768,NH=12,HD=64).

Sharding: 8 cores = 4 batches x 2 head-groups (6 heads each).

Key structure (v2):
- Mask compaction: attention_mask zeroes ~50% of k columns EXACTLY
  (exp(s-1e10) == 0 in f32), so the host gathers the unmasked k positions
  (n_b ~ 485..541 for the fixed inputs) and pads to K=640. Scores / exp /
  PV / rel DMA all shrink by ~40%. Pad k rows carry rel = -15000 so
  exp(score) == 0 exactly; no mask tensor on device at all.
- Host does layout-only prep: hs pre-transposed (hsT), hs compacted for
  k/v (hskvT), rel_pos/rel_2d_pos gathered along k and transposed to
  [k, q] layout, all bf16. All arithmetic (projections, rel add, exp,
  matmuls, softmax normalization) stays on device.
- Scores computed TRANSPOSED ([k,q]): per 128-k chunk, r12 = r1+r2 (DVE)
  is injected into PSUM via ONE identity matmul (start=True), then the
  zero-padded-head kTz @ qT matmul accumulates on top (stop=True).
  exp has no bias and reads PSUM directly.
- Softmax denominator from the [V|ones] PV matmul; context transposed
  back per 128-q block on PE, scaled by the reciprocal on DVE, output
  accumulated in SBUF and written as two large bf16 DMAs in a
  hardware-friendly [qh, i, p, cols] layout the host un-permutes.
"""
import os
import numpy as np
import ml_dtypes

from concourse import bacc, mybir, tile
from concourse.bass_utils import run_bass_kernel_spmd
from concourse.masks import make_identity

B, S, H = 4, 1024, 768
NH, HD = 12, 64
N_CORES = 8
HPC = 6            # heads per core
COLS = HPC * HD    # 384 output columns per core
KC = H // 128      # 6 contraction chunks for projections
K = 640            # compacted+padded k length
SC = K // 128      # 5 k chunks
QH = 2             # q halves of 512
PAD_REL = -15000.0  # per-rel pad value; r1+r2 = -30000 -> exp == 0
bf16 = mybir.dt.bfloat16
f32 = mybir.dt.float32
i32 = mybir.dt.int32
AF = mybir.ActivationFunctionType
BF16_NP = ml_dtypes.bfloat16

_compiled = None
last_result = None  # BassKernelResults of the most recent run (for test harness)


def _build():
    nc = bacc.Bacc("TRN2", target_bir_lowering=False, debug=False,
                   num_devices=N_CORES)
    hsT = nc.dram_tensor("hsT", [H, S], bf16, kind="ExternalInput").ap()
    hskvT = nc.dram_tensor("hskvT", [H, K], bf16, kind="ExternalInput").ap()
    wq = nc.dram_tensor("wq", [H, COLS], bf16, kind="ExternalInput").ap()
    wk = nc.dram_tensor("wk", [H, COLS], bf16, kind="ExternalInput").ap()
    wv = nc.dram_tensor("wv", [H, COLS], bf16, kind="ExternalInput").ap()
    bq = nc.dram_tensor("bq", [COLS], f32, kind="ExternalInput").ap()
    bk = nc.dram_tensor("bk", [COLS], f32, kind="ExternalInput").ap()
    bv = nc.dram_tensor("bv", [COLS], f32, kind="ExternalInput").ap()
    rel1 = nc.dram_tensor("rel1", [HPC, K, S], bf16, kind="ExternalInput").ap()
    rel2 = nc.dram_tensor("rel2", [HPC, K, S], bf16, kind="ExternalInput").ap()
    # out[qh, p, i, c] = ctx[qh*512 + i*128 + p, c]
    out = nc.dram_tensor("out", [QH, 128, 4, COLS], bf16,
                         kind="ExternalOutput").ap()

    with tile.TileContext(nc) as tc:
        with tc.tile_pool(name="const", bufs=1) as const, \
             tc.tile_pool(name="hst", bufs=1) as hst_pool, \
             tc.tile_pool(name="w", bufs=1) as w_pool, \
             tc.tile_pool(name="qk", bufs=1) as qk_pool, \
             tc.tile_pool(name="v", bufs=1) as v_pool, \
             tc.tile_pool(name="r1", bufs=2) as r1_pool, \
             tc.tile_pool(name="r2", bufs=2) as r2_pool, \
             tc.tile_pool(name="r12", bufs=3) as r12_pool, \
             tc.tile_pool(name="et", bufs=3) as e_pool, \
             tc.tile_pool(name="ctxt", bufs=2) as ctxt_pool, \
             tc.tile_pool(name="ob", bufs=1) as ob_pool:

            # ---- startup DMAs spread over the 3 issue-capable queues
            # (sync / scalar / gpsimd), each ordered by first use. The
            # scalar+gpsimd issues cost ~1us each, so no single queue may
            # serialize the critical path. ----
            import concourse.bass as bass
            hsT_sb = hst_pool.tile([128, KC, S], bf16)
            _hsT_r = hsT.rearrange("(c p) q -> p c q", p=128)
            hskvT_sb = hst_pool.tile([128, KC, K], bf16)
            _hskvT_r = hskvT.rearrange("(c p) q -> p c q", p=128)
            wq_sb = w_pool.tile([128, KC, COLS], bf16)
            wk_sb = w_pool.tile([128, KC, COLS], bf16)
            wv_sb = w_pool.tile([128, KC, COLS], bf16)
            _wq_r = wq.rearrange("(c p) n -> p c n", p=128)
            _wk_r = wk.rearrange("(c p) n -> p c n", p=128)

            # gpsimd ring: qproj/vproj weights
            nc.gpsimd.dma_start(out=wq_sb[:, :, 0:128], in_=_wq_r[:, :, 0:128])
            nc.gpsimd.dma_start(out=wv_sb, in_=wv.rearrange("(c p) n -> p c n", p=128))
            for c2 in range(3):
                nc.gpsimd.dma_start(out=hskvT_sb[:, c2 * 2:(c2 + 1) * 2, :],
                                    in_=_hskvT_r[:, c2 * 2:(c2 + 1) * 2, :])
            bv_bc = bass.AP(tensor=bv.tensor, offset=bv.offset,
                            ap=[[0, 128]] + list(bv.ap))
            bv_sb = const.tile([128, COLS], f32)
            nc.gpsimd.dma_start(out=bv_sb, in_=bv_bc)
            nc.gpsimd.dma_start(out=wq_sb[:, :, 128:384], in_=_wq_r[:, :, 128:384])

            # scalar ring: biases, hsT, kproj weights
            bq_sb = const.tile([128, 3], f32)
            nc.scalar.dma_start(out=bq_sb, in_=bq.rearrange("(c p) -> p c", p=128))
            bk_sb = const.tile([128, 3], f32)
            nc.scalar.dma_start(out=bk_sb, in_=bk.rearrange("(c p) -> p c", p=128))
            for c2 in range(3):
                nc.scalar.dma_start(out=hsT_sb[:, c2 * 2:(c2 + 1) * 2, :],
                                    in_=_hsT_r[:, c2 * 2:(c2 + 1) * 2, :])
            nc.scalar.dma_start(out=wk_sb[:, :, 0:128], in_=_wk_r[:, :, 0:128])
            nc.scalar.dma_start(out=wk_sb[:, :, 128:384], in_=_wk_r[:, :, 128:384])

            ident_f32 = const.tile([128, 128], f32)
            make_identity(nc, ident_f32)
            ident_b = const.tile([128, 128], bf16)
            nc.vector.tensor_copy(ident_b, ident_f32)

            _psum_cms = [tc.tile_pool(name="psA", bufs=2, space="PSUM"),
                         tc.tile_pool(name="psS", bufs=3, space="PSUM"),
                         tc.tile_pool(name="psV", bufs=2, space="PSUM"),
                         tc.tile_pool(name="psT", bufs=1, space="PSUM")]
            proj_psum, sc_psum, pv_psum, pt_psum = (cm.__enter__()
                                                    for cm in _psum_cms)

            # HAM warmup: dependency-free matmuls run during the startup DMA
            # window, flipping the PE clock gate to 2.4GHz before the real
            # projections arrive.
            garbage = const.tile([128, 384], bf16)
            nc.vector.memset(garbage, 0.0)
            def warm_burst(n):
                warm = sc_psum.tile([128, 512], f32, tag="ps")
                for _ in range(n):
                    nc.tensor.matmul(warm[:, 0:256], garbage[:, 0:128],
                                     garbage[:, 128:384], start=True, stop=True)
            warm_burst(18)

            # qT: [d(2 heads stacked), q] per head-pair hp; q scaled by 1/8.
            # kTz zero-padded per head (kTz[:, hp, hi]: head hi's 64 d-rows
            # live at their stacked position, other 64 rows are 0) so the
            # scores matmul streams a full 128-partition lhsT.
            qT = qk_pool.tile([128, 3, S], bf16)
            kTz = qk_pool.tile([128, 3, 2, K], bf16)
            nc.vector.memset(kTz, 0.0)
            v_sb = v_pool.tile([128, SC, HPC, HD + 1], bf16)
            nc.gpsimd.memset(v_sb[:, :, :, HD], 1.0)
            out_sb = ob_pool.tile([128, QH, 4, COLS], bf16)

            def emit_proj_q(hp):
                for sh in range(QH):
                    ssl = slice(sh * 512, (sh + 1) * 512)
                    psq = proj_psum.tile([128, 512], f32, tag="proj")
                    for c in range(KC):
                        nc.tensor.matmul(psq, wq_sb[:, c, hp * 128:(hp + 1) * 128],
                                         hsT_sb[:, c, ssl],
                                         start=(c == 0), stop=(c == KC - 1))
                    nc.scalar.activation(out=qT[:, hp, ssl], in_=psq, func=AF.Identity,
                                         bias=bq_sb[:, hp:hp + 1], scale=0.125)

            def emit_proj_k(hp):
                for part, ksl in ((0, slice(0, 512)), (1, slice(512, K))):
                    kw = 512 if part == 0 else K - 512
                    psk = proj_psum.tile([128, 512], f32, tag="proj")
                    for c in range(KC):
                        nc.tensor.matmul(psk[:, 0:kw],
                                         wk_sb[:, c, hp * 128:(hp + 1) * 128],
                                         hskvT_sb[:, c, ksl],
                                         start=(c == 0), stop=(c == KC - 1))
                    nc.vector.tensor_scalar_add(kTz[0:64, hp, 0, ksl],
                                                psk[0:64, 0:kw],
                                                bk_sb[0:64, hp:hp + 1])
                    nc.vector.tensor_scalar_add(kTz[64:128, hp, 1, ksl],
                                                psk[64:128, 0:kw],
                                                bk_sb[64:128, hp:hp + 1])

            def emit_proj_v(scs):
                for sc in scs:
                    psv_full = proj_psum.tile([128, 512], f32, tag="proj")
                    psv = psv_full[:, 0:384]
                    for c in range(KC):
                        nc.tensor.matmul(psv, hskvT_sb[:, c, sc * 128:(sc + 1) * 128],
                                         wv_sb[:, c, :],
                                         start=(c == 0), stop=(c == KC - 1))
                    nc.vector.tensor_add(
                        v_sb[:, sc, :, 0:HD],
                        psv.rearrange("p (h d) -> p h d", h=HPC),
                        bv_sb.rearrange("p (h d) -> p h d", h=HPC))

            def emit_rel_load(h, qh, split=False):
                # per-(head, q-half) loads so unit (h, qh) only waits on its
                # own 1.3MB, not the whole head's 2.6MB. split=True loads the
                # first 2 k-chunks separately so the first scores/preadd only
                # wait on ~1MB (startup units).
                qsl = slice(qh * 512, (qh + 1) * 512)
                _r1 = rel1[h].rearrange("(c p) q -> p c q", p=128)[:, :, qsl]
                _r2 = rel2[h].rearrange("(c p) q -> p c q", p=128)[:, :, qsl]
                r1t = r1_pool.tile([128, SC, 512], bf16, tag="r1")
                r2t = r2_pool.tile([128, SC, 512], bf16, tag="r2")
                if split:
                    nc.sync.dma_start(out=r1t[:, 0:2, :], in_=_r1[:, 0:2, :])
                    nc.sync.dma_start(out=r2t[:, 0:2, :], in_=_r2[:, 0:2, :])
                    nc.sync.dma_start(out=r1t[:, 2:SC, :], in_=_r1[:, 2:SC, :])
                    nc.sync.dma_start(out=r2t[:, 2:SC, :], in_=_r2[:, 2:SC, :])
                else:
                    nc.sync.dma_start(out=r1t, in_=_r1)
                    nc.sync.dma_start(out=r2t, in_=_r2)
                return (r1t, r2t)

            def emit_preadd(rel_t, split=False):
                r1t, r2t = rel_t
                r12 = r12_pool.tile([128, SC, 512], bf16, tag="r12")
                if split:
                    nc.vector.tensor_add(r12[:, 0:2, :], r1t[:, 0:2, :],
                                         r2t[:, 0:2, :])
                    nc.vector.tensor_add(r12[:, 2:SC, :], r1t[:, 2:SC, :],
                                         r2t[:, 2:SC, :])
                else:
                    nc.vector.tensor_add(r12, r1t, r2t)
                return r12

            def emit_attn(h, qh, r12):
                hp, hi = divmod(h, 2)
                qsl = slice(qh * 512, (qh + 1) * 512)
                et = e_pool.tile([128, SC, 512], bf16, tag="et")
                for kc in range(SC):
                    ps = sc_psum.tile([128, 512], f32, tag="ps")
                    nc.tensor.matmul(ps, ident_b, r12[:, kc, :],
                                     start=True, stop=False)
                    nc.tensor.matmul(ps, kTz[:, hp, hi, kc * 128:(kc + 1) * 128],
                                     qT[:, hp, qsl], start=False, stop=True)
                    nc.scalar.activation(out=et[:, kc, :], in_=ps, func=AF.Exp)

                pv = pv_psum.tile([HD + 1, 512], f32, tag="pv")
                for kc in range(SC):
                    nc.tensor.matmul(pv, v_sb[:, kc, h, :], et[:, kc, :],
                                     start=(kc == 0), stop=(kc == SC - 1))
                ctxT = ctxt_pool.tile([HD + 1, 512], bf16, tag="ctxT")
                nc.vector.tensor_copy(ctxT, pv)
                return (h, qh, ctxT)

            def emit_attn_out(state):
                h, qh, ctxT = state
                pt = pt_psum.tile([128, 4, HD + 2], bf16, tag="pt")
                for i in range(4):
                    nc.tensor.matmul(pt[:, i, 0:HD + 1], ctxT[:, i * 128:(i + 1) * 128],
                                     ident_b[:HD + 1, :HD + 1],
                                     is_transpose=True, start=True, stop=True)
                rec = ob_pool.tile([128, 4], f32, tag="rec")
                nc.vector.reciprocal(rec, pt[:, :, HD])
                rec_bc = bass.AP(tensor=rec.tensor, offset=rec.offset,
                                 ap=[list(rec.ap[0]), [rec.ap[1][0], 4], [0, HD]])
                nc.vector.tensor_mul(out_sb[:, qh, :, h * HD:(h + 1) * HD],
                                      pt[:, :, 0:HD], rec_bc)

            # interleave projections with attention so PE never drains;
            # rel loads run two units ahead, preadds one unit ahead,
            # out-transposes one unit behind.
            units = [(h, qh) for h in range(HPC) for qh in range(QH)]
            pending = []
            rel_q = []
            r12_q = []

            def run_unit(idx):
                if idx + 2 < len(units):
                    rel_q.append(emit_rel_load(*units[idx + 2]))
                if idx + 1 < len(units):
                    r12_q.append(emit_preadd(rel_q.pop(0)))
                st = emit_attn(*units[idx], r12_q.pop(0))
                if pending:
                    emit_attn_out(pending.pop())
                pending.append(st)

            rel_q.append(emit_rel_load(*units[0], split=True))
            rel_q.append(emit_rel_load(*units[1]))
            emit_proj_q(0)
            warm_burst(6)
            emit_proj_v(range(SC))
            warm_burst(6)
            emit_proj_k(0)
            r12_q.append(emit_preadd(rel_q.pop(0), split=True))
            run_unit(0)
            emit_proj_q(1)
            emit_proj_k(1)
            run_unit(1)
            run_unit(2)
            emit_proj_q(2)
            emit_proj_k(2)
            for idx in range(3, len(units)):
                run_unit(idx)
            nc.sync.dma_start(out=out[0], in_=out_sb[:, 0, :, :])
            emit_attn_out(pending.pop())
            nc.sync.dma_start(out=out[1], in_=out_sb[:, 1, :, :])

            for cm in reversed(_psum_cms):
                cm.__exit__(None, None, None)

    nc.compile()
    return nc


def _get_compiled():
    global _compiled
    if _compiled is None:
        _compiled = _build()
    return _compiled


def kernel(hidden_states, Wq, bq, Wk, bk, Wv, bv, rel_pos, rel_2d_pos,
           attention_mask, _trace=False):
    global last_result
    nc = _get_compiled()

    hidden_states = np.asarray(hidden_states, np.float32)
    Wq, Wk, Wv = (np.asarray(w, np.float32) for w in (Wq, Wk, Wv))
    bq, bk, bv = (np.asarray(x, np.float32) for x in (bq, bk, bv))
    rel_pos = np.asarray(rel_pos, np.float32)
    rel_2d_pos = np.asarray(rel_2d_pos, np.float32)
    attention_mask = np.asarray(attention_mask, np.int32)

    # per-batch unmasked k indices (mask==0 attends; mask==1 scores -1e10
    # -> exp == 0 exactly, so masked columns are dropped exactly)
    idxs = []
    for b in range(B):
        idx = np.nonzero(attention_mask[b, 0, 0] == 0)[0]
        assert len(idx) <= K, f"unmasked count {len(idx)} exceeds K={K}"
        idxs.append(idx)

    in_maps = []
    for c in range(N_CORES):
        b, hg = divmod(c, 2)
        cs = slice(hg * COLS, (hg + 1) * COLS)
        h0 = hg * HPC
        idx = idxs[b]
        n = len(idx)

        hsT_np = np.ascontiguousarray(hidden_states[b].T).astype(BF16_NP)
        hskvT_np = np.zeros((H, K), BF16_NP)
        hskvT_np[:, :n] = hidden_states[b][idx].T
        rel1_np = np.full((HPC, K, S), PAD_REL, BF16_NP)
        rel2_np = np.full((HPC, K, S), PAD_REL, BF16_NP)
        for j in range(HPC):
            rel1_np[j, :n] = rel_pos[b, h0 + j][:, idx].T
            rel2_np[j, :n] = rel_2d_pos[b, h0 + j][:, idx].T

        in_maps.append({
            "hsT": hsT_np,
            "hskvT": hskvT_np,
            "wq": Wq[:, cs].astype(BF16_NP),
            "wk": Wk[:, cs].astype(BF16_NP),
            "wv": Wv[:, cs].astype(BF16_NP),
            "bq": np.ascontiguousarray(bq[cs]) * np.float32(0.125),
            "bk": np.ascontiguousarray(bk[cs]),
            "bv": np.ascontiguousarray(bv[cs]),
            "rel1": rel1_np,
            "rel2": rel2_np,
        })

    kwargs = {}
    if _trace or os.environ.get("KERNEL_TRACE"):
        kwargs["trace"] = True
    last_result = run_bass_kernel_spmd(nc, in_maps, list(range(N_CORES)), **kwargs)

    result = np.empty((B, S, H), np.float32)
    for c in range(N_CORES):
        b, hg = divmod(c, 2)
        # out[qh, p, i, c] -> ctx[qh*512 + i*128 + p, c]
        o = np.asarray(last_result.results[c]["out"], BF16_NP).astype(np.float32)
        o = o.transpose(0, 2, 1, 3).reshape(S, COLS)
        result[b, :, hg * COLS:(hg + 1) * COLS] = o
    return result


# revision 22
# speedup vs baseline: 1.1554x; 1.0100x over previous
"""Trainium2 Bass kernel for ErnieLayout self-attention (B=4,S=1024,H=768,NH=12,HD=64).

Sharding: 8 cores = 4 batches x 2 head-groups (6 heads each).

Key structure (v2):
- Mask compaction: attention_mask zeroes ~50% of k columns EXACTLY
  (exp(s-1e10) == 0 in f32), so the host gathers the unmasked k positions
  (n_b ~ 485..541 for the fixed inputs) and pads to K=640. Scores / exp /
  PV / rel DMA all shrink by ~40%. Pad k rows carry rel = -15000 so
  exp(score) == 0 exactly; no mask tensor on device at all.
- Host does layout-only prep: hs pre-transposed (hsT), hs compacted for
  k/v (hskvT), rel_pos/rel_2d_pos gathered along k and transposed to
  [k, q] layout, all bf16. All arithmetic (projections, rel add, exp,
  matmuls, softmax normalization) stays on device.
- Scores computed TRANSPOSED ([k,q]): per 128-k chunk, r12 = r1+r2 (DVE)
  is injected into PSUM via ONE identity matmul (start=True), then the
  zero-padded-head kTz @ qT matmul accumulates on top (stop=True).
  exp has no bias and reads PSUM directly.
- Softmax denominator from the [V|ones] PV matmul; context transposed
  back per 128-q block on PE, scaled by the reciprocal on DVE, output
  accumulated in SBUF and written as two large bf16 DMAs in a
  hardware-friendly [qh, i, p, cols] layout the host un-permutes.
"""
import os
import numpy as np
import ml_dtypes

from concourse import bacc, mybir, tile
from concourse.bass_utils import run_bass_kernel_spmd
from concourse.masks import make_identity

B, S, H = 4, 1024, 768
NH, HD = 12, 64
N_CORES = 8
HPC = 6            # heads per core
COLS = HPC * HD    # 384 output columns per core
KC = H // 128      # 6 contraction chunks for projections
K = 640            # compacted+padded k length
SC = K // 128      # 5 k chunks
QH = 2             # q halves of 512
PAD_REL = -15000.0  # per-rel pad value; r1+r2 = -30000 -> exp == 0
bf16 = mybir.dt.bfloat16
f32 = mybir.dt.float32
i32 = mybir.dt.int32
AF = mybir.ActivationFunctionType
BF16_NP = ml_dtypes.bfloat16

_compiled = None
last_result = None  # BassKernelResults of the most recent run (for test harness)


def _build():
    nc = bacc.Bacc("TRN2", target_bir_lowering=False, debug=False,
                   num_devices=N_CORES)
    hsT = nc.dram_tensor("hsT", [H, S], bf16, kind="ExternalInput").ap()
    hskvT = nc.dram_tensor("hskvT", [H, K], bf16, kind="ExternalInput").ap()
    wq = nc.dram_tensor("wq", [H, COLS], bf16, kind="ExternalInput").ap()
    wk = nc.dram_tensor("wk", [H, COLS], bf16, kind="ExternalInput").ap()
    wv = nc.dram_tensor("wv", [H, COLS], bf16, kind="ExternalInput").ap()
    bq = nc.dram_tensor("bq", [COLS], f32, kind="ExternalInput").ap()
    bk = nc.dram_tensor("bk", [COLS], f32, kind="ExternalInput").ap()
    bv = nc.dram_tensor("bv", [COLS], f32, kind="ExternalInput").ap()
    rel1 = nc.dram_tensor("rel1", [HPC, K, S], bf16, kind="ExternalInput").ap()
    rel2 = nc.dram_tensor("rel2", [HPC, K, S], bf16, kind="ExternalInput").ap()
    # out[qh, p, i, c] = ctx[qh*512 + i*128 + p, c]
    out = nc.dram_tensor("out", [QH, 128, 4, COLS], bf16,
                         kind="ExternalOutput").ap()

    with tile.TileContext(nc) as tc:
        with tc.tile_pool(name="const", bufs=1) as const, \
             tc.tile_pool(name="hst", bufs=1) as hst_pool, \
             tc.tile_pool(name="w", bufs=1) as w_pool, \
             tc.tile_pool(name="qk", bufs=1) as qk_pool, \
             tc.tile_pool(name="v", bufs=1) as v_pool, \
             tc.tile_pool(name="r1", bufs=2) as r1_pool, \
             tc.tile_pool(name="r2", bufs=2) as r2_pool, \
             tc.tile_pool(name="r12", bufs=3) as r12_pool, \
             tc.tile_pool(name="et", bufs=3) as e_pool, \
             tc.tile_pool(name="ctxt", bufs=2) as ctxt_pool, \
             tc.tile_pool(name="ob", bufs=1) as ob_pool:

            # ---- startup DMAs spread over the 3 issue-capable queues
            # (sync / scalar / gpsimd), each ordered by first use. The
            # scalar+gpsimd issues cost ~1us each, so no single queue may
            # serialize the critical path. ----
            import concourse.bass as bass
            hsT_sb = hst_pool.tile([128, KC, S], bf16)
            _hsT_r = hsT.rearrange("(c p) q -> p c q", p=128)
            hskvT_sb = hst_pool.tile([128, KC, K], bf16)
            _hskvT_r = hskvT.rearrange("(c p) q -> p c q", p=128)
            wq_sb = w_pool.tile([128, KC, COLS], bf16)
            wk_sb = w_pool.tile([128, KC, COLS], bf16)
            wv_sb = w_pool.tile([128, KC, COLS], bf16)
            _wq_r = wq.rearrange("(c p) n -> p c n", p=128)
            _wk_r = wk.rearrange("(c p) n -> p c n", p=128)

            # gpsimd ring: kproj/vproj inputs first, then the rest
            nc.gpsimd.dma_start(out=wk_sb[:, :, 0:128], in_=_wk_r[:, :, 0:128])
            for c2 in range(3):
                nc.gpsimd.dma_start(out=hskvT_sb[:, c2 * 2:(c2 + 1) * 2, :],
                                    in_=_hskvT_r[:, c2 * 2:(c2 + 1) * 2, :])
            nc.gpsimd.dma_start(out=wv_sb, in_=wv.rearrange("(c p) n -> p c n", p=128))
            bv_bc = bass.AP(tensor=bv.tensor, offset=bv.offset,
                            ap=[[0, 128]] + list(bv.ap))
            bv_sb = const.tile([128, COLS], f32)
            nc.gpsimd.dma_start(out=bv_sb, in_=bv_bc)
            nc.gpsimd.dma_start(out=wq_sb[:, :, 128:384], in_=_wq_r[:, :, 128:384])
            nc.gpsimd.dma_start(out=wk_sb[:, :, 128:384], in_=_wk_r[:, :, 128:384])

            # scalar ring: qproj inputs
            nc.scalar.dma_start(out=wq_sb[:, :, 0:128], in_=_wq_r[:, :, 0:128])
            bq_sb = const.tile([128, 3], f32)
            nc.scalar.dma_start(out=bq_sb, in_=bq.rearrange("(c p) -> p c", p=128))
            bk_sb = const.tile([128, 3], f32)
            nc.scalar.dma_start(out=bk_sb, in_=bk.rearrange("(c p) -> p c", p=128))
            for c2 in range(3):
                nc.scalar.dma_start(out=hsT_sb[:, c2 * 2:(c2 + 1) * 2, :],
                                    in_=_hsT_r[:, c2 * 2:(c2 + 1) * 2, :])

            ident_f32 = const.tile([128, 128], f32)
            make_identity(nc, ident_f32)
            ident_b = const.tile([128, 128], bf16)
            nc.vector.tensor_copy(ident_b, ident_f32)

            _psum_cms = [tc.tile_pool(name="psA", bufs=2, space="PSUM"),
                         tc.tile_pool(name="psS", bufs=3, space="PSUM"),
                         tc.tile_pool(name="psV", bufs=2, space="PSUM"),
                         tc.tile_pool(name="psT", bufs=1, space="PSUM")]
            proj_psum, sc_psum, pv_psum, pt_psum = (cm.__enter__()
                                                    for cm in _psum_cms)

            # HAM warmup: dependency-free matmuls run during the startup DMA
            # window, flipping the PE clock gate to 2.4GHz before the real
            # projections arrive.
            garbage = const.tile([128, 384], bf16)
            nc.vector.memset(garbage, 0.0)
            def warm_burst(n):
                warm = sc_psum.tile([128, 512], f32, tag="ps")
                for _ in range(n):
                    nc.tensor.matmul(warm[:, 0:256], garbage[:, 0:128],
                                     garbage[:, 128:384], start=True, stop=True)
            warm_burst(18)

            # qT: [d(2 heads stacked), q] per head-pair hp; q scaled by 1/8.
            # kTz zero-padded per head (kTz[:, hp, hi]: head hi's 64 d-rows
            # live at their stacked position, other 64 rows are 0) so the
            # scores matmul streams a full 128-partition lhsT.
            qT = qk_pool.tile([128, 3, S], bf16)
            kTz = qk_pool.tile([128, 3, 2, K], bf16)
            nc.vector.memset(kTz, 0.0)
            v_sb = v_pool.tile([128, SC, HPC, HD + 1], bf16)
            nc.gpsimd.memset(v_sb[:, :, :, HD], 1.0)
            out_sb = ob_pool.tile([128, QH, 4, COLS], bf16)

            def emit_proj_q(hp):
                for sh in range(QH):
                    ssl = slice(sh * 512, (sh + 1) * 512)
                    psq = proj_psum.tile([128, 512], f32, tag="proj")
                    for c in range(KC):
                        nc.tensor.matmul(psq, wq_sb[:, c, hp * 128:(hp + 1) * 128],
                                         hsT_sb[:, c, ssl],
                                         start=(c == 0), stop=(c == KC - 1))
                    nc.scalar.activation(out=qT[:, hp, ssl], in_=psq, func=AF.Identity,
                                         bias=bq_sb[:, hp:hp + 1], scale=0.125)

            def emit_proj_k(hp):
                for part, ksl in ((0, slice(0, 512)), (1, slice(512, K))):
                    kw = 512 if part == 0 else K - 512
                    psk = proj_psum.tile([128, 512], f32, tag="proj")
                    for c in range(KC):
                        nc.tensor.matmul(psk[:, 0:kw],
                                         wk_sb[:, c, hp * 128:(hp + 1) * 128],
                                         hskvT_sb[:, c, ksl],
                                         start=(c == 0), stop=(c == KC - 1))
                    nc.scalar.activation(out=kTz[0:64, hp, 0, ksl],
                                         in_=psk[0:64, 0:kw], func=AF.Identity,
                                         bias=bk_sb[0:64, hp:hp + 1], scale=1.0)
                    nc.scalar.activation(out=kTz[64:128, hp, 1, ksl],
                                         in_=psk[64:128, 0:kw], func=AF.Identity,
                                         bias=bk_sb[64:128, hp:hp + 1], scale=1.0)

            def emit_proj_v(scs):
                for sc in scs:
                    psv_full = proj_psum.tile([128, 512], f32, tag="proj")
                    psv = psv_full[:, 0:384]
                    for c in range(KC):
                        nc.tensor.matmul(psv, hskvT_sb[:, c, sc * 128:(sc + 1) * 128],
                                         wv_sb[:, c, :],
                                         start=(c == 0), stop=(c == KC - 1))
                    nc.vector.tensor_add(
                        v_sb[:, sc, :, 0:HD],
                        psv.rearrange("p (h d) -> p h d", h=HPC),
                        bv_sb.rearrange("p (h d) -> p h d", h=HPC))

            def emit_rel_load(h, qh, split=False):
                # per-(head, q-half) loads so unit (h, qh) only waits on its
                # own 1.3MB, not the whole head's 2.6MB. split=True loads the
                # first 2 k-chunks separately so the first scores/preadd only
                # wait on ~1MB (startup units).
                qsl = slice(qh * 512, (qh + 1) * 512)
                _r1 = rel1[h].rearrange("(c p) q -> p c q", p=128)[:, :, qsl]
                _r2 = rel2[h].rearrange("(c p) q -> p c q", p=128)[:, :, qsl]
                r1t = r1_pool.tile([128, SC, 512], bf16, tag="r1")
                r2t = r2_pool.tile([128, SC, 512], bf16, tag="r2")
                if split:
                    nc.sync.dma_start(out=r1t[:, 0:2, :], in_=_r1[:, 0:2, :])
                    nc.sync.dma_start(out=r2t[:, 0:2, :], in_=_r2[:, 0:2, :])
                    nc.sync.dma_start(out=r1t[:, 2:SC, :], in_=_r1[:, 2:SC, :])
                    nc.sync.dma_start(out=r2t[:, 2:SC, :], in_=_r2[:, 2:SC, :])
                else:
                    nc.sync.dma_start(out=r1t, in_=_r1)
                    nc.sync.dma_start(out=r2t, in_=_r2)
                return (r1t, r2t)

            def emit_preadd(rel_t, split=False):
                r1t, r2t = rel_t
                r12 = r12_pool.tile([128, SC, 512], bf16, tag="r12")
                if split:
                    nc.vector.tensor_add(r12[:, 0:2, :], r1t[:, 0:2, :],
                                         r2t[:, 0:2, :])
                    nc.vector.tensor_add(r12[:, 2:SC, :], r1t[:, 2:SC, :],
                                         r2t[:, 2:SC, :])
                else:
                    nc.vector.tensor_add(r12, r1t, r2t)
                return r12

            def emit_scores(h, qh, r12):
                hp, hi = divmod(h, 2)
                qsl = slice(qh * 512, (qh + 1) * 512)
                et = e_pool.tile([128, SC, 512], bf16, tag="et")
                for kc in range(SC):
                    ps = sc_psum.tile([128, 512], f32, tag="ps")
                    nc.tensor.matmul(ps, ident_b, r12[:, kc, :],
                                     start=True, stop=False)
                    nc.tensor.matmul(ps, kTz[:, hp, hi, kc * 128:(kc + 1) * 128],
                                     qT[:, hp, qsl], start=False, stop=True)
                    nc.scalar.activation(out=et[:, kc, :], in_=ps, func=AF.Exp)
                return (h, qh, et)

            def emit_pv(state):
                h, qh, et = state
                pv = pv_psum.tile([HD + 1, 512], f32, tag="pv")
                for kc in range(SC):
                    nc.tensor.matmul(pv, v_sb[:, kc, h, :], et[:, kc, :],
                                     start=(kc == 0), stop=(kc == SC - 1))
                ctxT = ctxt_pool.tile([HD + 1, 512], bf16, tag="ctxT")
                nc.vector.tensor_copy(ctxT, pv)
                return (h, qh, ctxT)

            def emit_attn(h, qh, r12):
                return emit_pv(emit_scores(h, qh, r12))

            def emit_attn_out(state):
                h, qh, ctxT = state
                pt = pt_psum.tile([128, 4, HD + 2], bf16, tag="pt")
                for i in range(4):
                    nc.tensor.matmul(pt[:, i, 0:HD + 1], ctxT[:, i * 128:(i + 1) * 128],
                                     ident_b[:HD + 1, :HD + 1],
                                     is_transpose=True, start=True, stop=True)
                rec = ob_pool.tile([128, 4], f32, tag="rec")
                nc.vector.reciprocal(rec, pt[:, :, HD])
                rec_bc = bass.AP(tensor=rec.tensor, offset=rec.offset,
                                 ap=[list(rec.ap[0]), [rec.ap[1][0], 4], [0, HD]])
                nc.vector.tensor_mul(out_sb[:, qh, :, h * HD:(h + 1) * HD],
                                      pt[:, :, 0:HD], rec_bc)

            # interleave projections with attention so PE never drains;
            # rel loads run two units ahead, preadds one unit ahead,
            # out-transposes one unit behind.
            units = [(h, qh) for h in range(HPC) for qh in range(QH)]
            pending = []
            rel_q = []
            r12_q = []

            def run_unit(idx):
                if idx + 2 < len(units):
                    rel_q.append(emit_rel_load(*units[idx + 2]))
                if idx + 1 < len(units):
                    r12_q.append(emit_preadd(rel_q.pop(0)))
                st = emit_attn(*units[idx], r12_q.pop(0))
                if pending:
                    emit_attn_out(pending.pop())
                pending.append(st)

            def run_unit2(idx):
                # preadd for this unit was already emitted; rel prefetch for
                # idx+2 was already issued by the caller
                if idx + 1 < len(units):
                    r12_q.append(emit_preadd(rel_q.pop(0)))
                st = emit_attn(*units[idx], r12_q.pop(0))
                if pending:
                    emit_attn_out(pending.pop())
                pending.append(st)

            rel_q.append(emit_rel_load(*units[0], split=True))
            rel_q.append(emit_rel_load(*units[1]))
            emit_proj_k(0)
            warm_burst(6)
            emit_proj_q(0)
            r12_q.append(emit_preadd(rel_q.pop(0), split=True))
            r12_q.append(emit_preadd(rel_q.pop(0)))
            warm_burst(6)
            rel_q.append(emit_rel_load(*units[2]))
            st0 = emit_scores(*units[0], r12_q.pop(0))
            emit_proj_v(range(SC))
            pending.append(emit_pv(st0))
            run_unit2(1)
            rel_q.append(emit_rel_load(*units[3]))
            emit_proj_q(1)
            emit_proj_k(1)
            run_unit(2)
            emit_proj_q(2)
            emit_proj_k(2)
            for idx in range(3, len(units)):
                run_unit(idx)
            nc.sync.dma_start(out=out[0], in_=out_sb[:, 0, :, :])
            emit_attn_out(pending.pop())
            nc.sync.dma_start(out=out[1], in_=out_sb[:, 1, :, :])

            for cm in reversed(_psum_cms):
                cm.__exit__(None, None, None)

    nc.compile()
    return nc


def _get_compiled():
    global _compiled
    if _compiled is None:
        _compiled = _build()
    return _compiled


def kernel(hidden_states, Wq, bq, Wk, bk, Wv, bv, rel_pos, rel_2d_pos,
           attention_mask, _trace=False):
    global last_result
    nc = _get_compiled()

    hidden_states = np.asarray(hidden_states, np.float32)
    Wq, Wk, Wv = (np.asarray(w, np.float32) for w in (Wq, Wk, Wv))
    bq, bk, bv = (np.asarray(x, np.float32) for x in (bq, bk, bv))
    rel_pos = np.asarray(rel_pos, np.float32)
    rel_2d_pos = np.asarray(rel_2d_pos, np.float32)
    attention_mask = np.asarray(attention_mask, np.int32)

    # per-batch unmasked k indices (mask==0 attends; mask==1 scores -1e10
    # -> exp == 0 exactly, so masked columns are dropped exactly)
    idxs = []
    for b in range(B):
        idx = np.nonzero(attention_mask[b, 0, 0] == 0)[0]
        assert len(idx) <= K, f"unmasked count {len(idx)} exceeds K={K}"
        idxs.append(idx)

    in_maps = []
    for c in range(N_CORES):
        b, hg = divmod(c, 2)
        cs = slice(hg * COLS, (hg + 1) * COLS)
        h0 = hg * HPC
        idx = idxs[b]
        n = len(idx)

        hsT_np = np.ascontiguousarray(hidden_states[b].T).astype(BF16_NP)
        hskvT_np = np.zeros((H, K), BF16_NP)
        hskvT_np[:, :n] = hidden_states[b][idx].T
        rel1_np = np.full((HPC, K, S), PAD_REL, BF16_NP)
        rel2_np = np.full((HPC, K, S), PAD_REL, BF16_NP)
        for j in range(HPC):
            rel1_np[j, :n] = rel_pos[b, h0 + j][:, idx].T
            rel2_np[j, :n] = rel_2d_pos[b, h0 + j][:, idx].T

        in_maps.append({
            "hsT": hsT_np,
            "hskvT": hskvT_np,
            "wq": Wq[:, cs].astype(BF16_NP),
            "wk": Wk[:, cs].astype(BF16_NP),
            "wv": Wv[:, cs].astype(BF16_NP),
            "bq": np.ascontiguousarray(bq[cs]) * np.float32(0.125),
            "bk": np.ascontiguousarray(bk[cs]),
            "bv": np.ascontiguousarray(bv[cs]),
            "rel1": rel1_np,
            "rel2": rel2_np,
        })

    kwargs = {}
    if _trace or os.environ.get("KERNEL_TRACE"):
        kwargs["trace"] = True
    last_result = run_bass_kernel_spmd(nc, in_maps, list(range(N_CORES)), **kwargs)

    result = np.empty((B, S, H), np.float32)
    for c in range(N_CORES):
        b, hg = divmod(c, 2)
        # out[qh, p, i, c] -> ctx[qh*512 + i*128 + p, c]
        o = np.asarray(last_result.results[c]["out"], BF16_NP).astype(np.float32)
        o = o.transpose(0, 2, 1, 3).reshape(S, COLS)
        result[b, :, hg * COLS:(hg + 1) * COLS] = o
    return result
